# revision 20
# baseline (speedup 1.0000x reference)
"""Multi-head attention kernel for Trainium2, data-parallel over batch on 8 NeuronCores.

Reference computation (per batch element b of 8):
    qkv = x @ W_qkv.T + b_qkv            [1024, 2304]
    q, k, v = split(qkv)                 each [1024, 768], 12 heads x 64
    S_h = q_h @ k_h.T * d**-0.5          [1024, 1024] per head
    A_h = softmax(S_h, axis=-1)
    o_h = A_h @ v_h                      [1024, 64]
    y = concat(o) @ W_out.T + b_out      [1024, 768]

Strategy: one batch element per core (zero communication). All attention
matmuls (S, A@V) and the v/out projections run in bf16 with f32 PSUM
accumulation. The q/k projection runs in fp8e4m3 DoubleRow: two 128-row
k-subtiles contract per instruction, halving the instruction count for its
K=768 contraction (measured: a DoubleRow matmul costs the same per output
column as bf16, so fp8 only pays where it deepens contraction — it would NOT
speed up the K=64 scores matmul). fp8 on the q/k path is safe: quantization
noise reaches the output only through softmax weight perturbations (~1%),
while v/A/out-proj quantization would hit the output linearly, so those stay
bf16.

Scores are computed transposed (S^T[j,i] keys-on-partition) so exp(S^T) feeds
A@V as the moving operand with V stationary. A ones-column appended to V
yields softmax denominators free. Softmax max-subtraction is skipped:
scores*scale are O(1) (std ~0.14); f32 exp cannot overflow below inputs of 88.

Scheduling: ACT exp takes ~1.1us per [128,1024] tile (96 tiles); the PE has
slightly more total work, so both engines must stay saturated. Attention runs
as one flat software pipeline over (head, kc): S(step+1) issues before
A@V(step) so the PE never waits on exp. PE bubbles take real work from a
filler queue (remaining v chunks, later q/k chunks, partial output projection
fc 0..3 flushed to SBUF f32). Extra fillers are pumped at head boundaries to
cover the head-drain DVE latency — a ~1us PE stall measurably drops the PE
DVFS clock from 2.4 GHz to ~1.2 GHz for tens of microseconds, which is the
dominant failure mode. An idempotent junk-work generator backstops the queue.
The denominator row is staged PSUM->SBUF before reciprocal_approx_fast (the
custom DVE op misreads PSUM rows at partition base 64); reciprocals stay f32
end-to-end. Normalize multiplies run on DVE (GpSimd TensorTensor measured
2.1us vs DVE 0.6us for [64,1024]).
"""

import numpy as np
import ml_dtypes

B, N, D, H, HD = 8, 1024, 768, 12, 64
NCORES = 8
SCALE = float(D) ** -0.5
DC = D // 128            # 6 chunks of 128 for d=768
IC = N // 128            # 8 token chunks
KC = N // 128            # 8 key chunks
NI = 3                   # 256-wide contraction double-chunks for fp8 qk-proj


def _build(has_bqkv: bool, has_bout: bool):
    import concourse.bass as bass
    import concourse.mybir as mybir
    import concourse.tile as tile
    from concourse import bacc

    f32 = mybir.dt.float32
    bf16 = mybir.dt.bfloat16
    fp8 = mybir.dt.float8e4
    Exp = mybir.ActivationFunctionType.Exp
    DoubleRow = mybir.MatmulPerfMode.DoubleRow

    nc = bacc.Bacc("TRN2", target_bir_lowering=False, debug=False,
                   num_devices=NCORES)

    xT_ext = nc.dram_tensor("xT", [D, N], bf16, kind="ExternalInput")
    xTf_ext = [nc.dram_tensor(f"xTf{i}", [128, 2 * N], fp8, kind="ExternalInput")
               for i in range(NI)]
    xTr_ext = [nc.dram_tensor(f"xTr{i}", [128, 2 * N], fp8, kind="ExternalInput")
               for i in range(NI)]
    wqf_ext = [nc.dram_tensor(f"wqf{i}", [128, 4 * D], fp8, kind="ExternalInput")
               for i in range(NI)]
    wvT_ext = nc.dram_tensor("wvT", [D, D], bf16, kind="ExternalInput")
    woutT_ext = nc.dram_tensor("woutT", [D, D], bf16, kind="ExternalInput")
    if has_bqkv:
        bqkv_ext = nc.dram_tensor("bqkv", [2 * D], f32, kind="ExternalInput")
        bv16_ext = nc.dram_tensor("bv16", [D], bf16, kind="ExternalInput")
    if has_bout:
        bout16_ext = nc.dram_tensor("bout16", [D], bf16, kind="ExternalInput")
    out_ext = nc.dram_tensor("out", [N, D], f32, kind="ExternalOutput")
    recip_dram = nc.dram_tensor("recip_scratch", [H, N], f32)

    with tile.TileContext(nc) as tc:
        with (
            tc.tile_pool(name="w", bufs=1) as wpool,
            tc.tile_pool(name="act", bufs=1) as apool,
            tc.tile_pool(name="es", bufs=7) as espool,
            tc.tile_pool(name="rows", bufs=2) as rowpool,
            tc.tile_pool(name="bc", bufs=3) as bcpool,
            tc.tile_pool(name="y", bufs=3) as ypool,
            tc.tile_pool(name="ps", bufs=1, space="PSUM") as pspool,
        ):
            # ---- input DMAs: qk-proj operands first for earliest PE start ----
            xTf = [wpool.tile([128, 2 * N], fp8, tag=f"xTf{i}", name=f"xTf{i}") for i in range(NI)]
            xTr = [wpool.tile([128, 2 * N], fp8, tag=f"xTr{i}", name=f"xTr{i}") for i in range(NI)]
            wqf = [wpool.tile([128, 4 * D], fp8, tag=f"wqf{i}", name=f"wqf{i}") for i in range(NI)]
            xT = [wpool.tile([128, N], bf16, tag=f"xT{i}", name=f"xT{i}") for i in range(DC)]
            wv = [wpool.tile([128, D], bf16, tag=f"wv{i}", name=f"wv{i}") for i in range(DC)]
            wo = [wpool.tile([128, D], bf16, tag=f"wo{i}", name=f"wo{i}") for i in range(DC)]
            for i in range(NI):
                nc.scalar.dma_start(out=xTf[i][:], in_=xTf_ext[i][:, :])
                nc.sync.dma_start(out=wqf[i][:], in_=wqf_ext[i][:, :])
            for i in range(NI):
                nc.gpsimd.dma_start(out=xTr[i][:], in_=xTr_ext[i][:, :])
            for dc in range(DC):
                nc.scalar.dma_start(out=xT[dc][:], in_=xT_ext[dc * 128:(dc + 1) * 128, :])
                nc.sync.dma_start(out=wv[dc][:], in_=wvT_ext[dc * 128:(dc + 1) * 128, :])
            for dc in range(DC):
                nc.scalar.dma_start(out=wo[dc][:], in_=woutT_ext[dc * 128:(dc + 1) * 128, :])

            # PSUM budget (8 banks): A,B = sps double-buffer (2+2), C = ot (2),
            # D = two [128,512] one-bank tiles for warmup/filler projections.
            def big_ps(tag, name):
                return pspool.tile([128, N], f32, tag=tag, name=name)

            def half_ps(name):
                return pspool.tile([128, 512], f32, tag="D", bufs=2, name=name)

            if has_bqkv:
                bqk_t = wpool.tile([128, 2 * DC], f32, tag="bqk")
                for jc in range(2 * DC):
                    nc.sync.dma_start(
                        out=bqk_t[:, jc:jc + 1],
                        in_=bqkv_ext[jc * 128:(jc + 1) * 128][:, None])
                bv_t = wpool.tile([1, D], bf16, tag="bv")
                nc.sync.dma_start(out=bv_t[:], in_=bv16_ext[:][None, :])
            if has_bout:
                bo_t = wpool.tile([1, D], bf16, tag="bo")
                nc.sync.dma_start(out=bo_t[:], in_=bout16_ext[:][None, :])
            if has_bqkv or has_bout:
                ones_t = wpool.tile([1, 128], bf16, tag="ones")
                nc.vector.memset(ones_t[:], 1.0)

            xTf3 = [t.rearrange("p (two n) -> p two n", two=2) for t in xTf]
            xTr3 = [t.rearrange("p (two n) -> p two n", two=2) for t in xTr]
            wqf3 = [t.rearrange("p (two n) -> p two n", two=2) for t in wqf]

            # ---- q^T/k^T fp8 DoubleRow tiles. Host W-permutation puts
            # [A0-31|B0-31|A32-63|B32-63] on each chunk's partitions, so the
            # DR [32p x 2slot] pack is one cast + one partition-shift DMA.
            qk8 = [apool.tile([128, 2, N], fp8, tag=f"qk8_{j}", name=f"qk8_{j}")
                   for j in range(2 * DC)]

            def qk_store(jc, src, s=0, e=N):
                t = qk8[jc]
                if has_bqkv:
                    nc.vector.tensor_scalar_add(t[:, 0, s:e], src,
                                                bqk_t[:, jc:jc + 1])
                else:
                    nc.vector.tensor_copy(t[:, 0, s:e], src)
                nc.gpsimd.dma_start(out=t[0:64, 1, s:e], in_=t[64:128, 0, s:e])

            def qk_chunk_full(jc, tag):
                """q/k projection chunk jc, fp8 DoubleRow + x-residual."""
                ps = big_ps(tag, f"qkps{jc}")
                for ih in range(2):
                    for i in range(NI):
                        for x3 in (xTf3, xTr3):
                            nc.tensor.matmul(
                                ps[:, ih * 512:(ih + 1) * 512],
                                wqf3[i][:, :, jc * 128:(jc + 1) * 128],
                                x3[i][:, :, ih * 512:(ih + 1) * 512],
                                start=(i == 0 and x3 is xTf3),
                                stop=(i == NI - 1 and x3 is xTr3),
                                perf_mode=DoubleRow)
                qk_store(jc, ps[:, 0:N])

            def gen_qk_chunk(jc):
                """Filler generator: one chunk as 2 half-bank units."""
                for ih in range(2):
                    ps = half_ps(f"qkh{jc}_{ih}")
                    for i in range(NI):
                        for x3 in (xTf3, xTr3):
                            yield nc.tensor.matmul(
                                ps[:], wqf3[i][:, :, jc * 128:(jc + 1) * 128],
                                x3[i][:, :, ih * 512:(ih + 1) * 512],
                                start=(i == 0 and x3 is xTf3),
                                stop=(i == NI - 1 and x3 is xTr3),
                                perf_mode=DoubleRow)
                    qk_store(jc, ps[:], s=ih * 512, e=(ih + 1) * 512)

            # ---- v : [1024 tokens, 12 heads x (64+1)] with ones column ----
            v = [apool.tile([128, H, HD + 1], bf16, tag=f"v{i}", name=f"v{i}") for i in range(IC)]
            for ic in range(IC):
                nc.vector.memset(v[ic][:, :, HD:HD + 1], 1.0)
            vsplits = [(0, 512), (512, 768)]

            def v_chunk(ic, tag):
                ps = big_ps(tag, f"vps{ic}")
                if has_bqkv:
                    for s, e in vsplits:
                        nc.tensor.matmul(ps[:, s:e], ones_t[:],
                                         bv_t[:, s:e], start=True, stop=False)
                for s, e in vsplits:
                    for dc in range(DC):
                        nc.tensor.matmul(
                            ps[:, s:e],
                            xT[dc][:, ic * 128:(ic + 1) * 128],
                            wv[dc][:, s:e],
                            start=(dc == 0 and not has_bqkv), stop=(dc == DC - 1))
                nc.vector.tensor_copy(
                    v[ic][:, :, 0:HD],
                    ps[:, 0:D].rearrange("p (h e) -> p h e", h=H))

            def gen_v_chunk(ic):
                """half-bank filler version: heads 0..7 then 8..11."""
                for s, e in vsplits:
                    ps = half_ps(f"vh{ic}_{s}")
                    if has_bqkv:
                        yield nc.tensor.matmul(ps[:, 0:e - s], ones_t[:],
                                               bv_t[:, s:e], start=True, stop=False)
                    for dc in range(DC):
                        yield nc.tensor.matmul(
                            ps[:, 0:e - s],
                            xT[dc][:, ic * 128:(ic + 1) * 128],
                            wv[dc][:, s:e],
                            start=(dc == 0 and not has_bqkv), stop=(dc == DC - 1))
                    h0, h1 = s // HD, e // HD
                    nc.vector.tensor_copy(
                        v[ic][:, h0:h1, 0:HD],
                        ps[:, 0:e - s].rearrange("p (h e) -> p h e", h=h1 - h0))


            # ---- attention: flat software pipeline over (head, kc) ----
            # otu: unnormalized o^T + denominator row per head, f32, short-lived
            otn = [apool.tile([128, N], bf16, tag=f"otn{t}", name=f"otn{t}") for t in range(DC)]
            otu = {}

            fillers = []

            def fill(n):
                while n > 0 and fillers:
                    try:
                        next(fillers[0])
                        n -= 1
                    except StopIteration:
                        fillers.pop(0)

            def smm(h, kc, sps):
                qt, kt = qk8[h // 2], qk8[DC + h // 2]
                p0 = 32 * (h % 2)
                for ih in range(2):
                    nc.tensor.matmul(
                        sps[:, ih * 512:(ih + 1) * 512],
                        kt[p0:p0 + 32, :, kc * 128:(kc + 1) * 128],
                        qt[p0:p0 + 32, :, ih * 512:(ih + 1) * 512],
                        start=True, stop=True, perf_mode=DoubleRow)

            def avmm(h, kc, et, ot):
                for ih in range(2):
                    nc.tensor.matmul(
                        ot[0:HD + 1, ih * 512:(ih + 1) * 512],
                        v[kc][:, h, :],
                        et[:, ih * 512:(ih + 1) * 512],
                        start=(kc == 0), stop=(kc == KC - 1))

            def head_epilogue(h, ot):
                # recip path first (it gates normalize/out-proj); the custom
                # DVE op misreads rows at partition base 64, so stage to a
                # base-0 SBUF row before reciprocal.
                drow = rowpool.tile([1, N], f32, tag="drow", bufs=2, name=f"drow{h}")
                nc.vector.tensor_copy(drow[:], ot[HD:HD + 1, :])
                rc32 = rowpool.tile([1, N], f32, tag="rc32", bufs=2, name=f"rc32_{h}")
                nc.vector.reciprocal_approx_fast(rc32[:], drow[:])
                eng = nc.sync if h % 2 == 0 else nc.scalar
                eng.dma_start(out=recip_dram[h:h + 1, :], in_=rc32[:])
                otu[h] = rowpool.tile([HD + 1, N], f32, tag="otu", bufs=3,
                                      name=f"otu{h}")
                nc.vector.tensor_copy(otu[h][:], ot[:HD + 1, :])

            def normalize_pair(t):
                for i in range(2):
                    h = 2 * t + i
                    bc = bcpool.tile([64, N], f32, tag="bc", bufs=3, name=f"bc{h}")
                    eng = nc.sync if h % 2 == 0 else nc.scalar
                    eng.dma_start(
                        out=bc[:],
                        in_=recip_dram[h:h + 1, :].to_broadcast((64, N)))
                    with nc.allow_low_precision(reason="bf16 normalized o; 2e-2 gate"):
                        nc.vector.tensor_mul(otn[t][64 * i:64 * i + 64, :],
                                             otu.pop(h)[0:HD, :], bc[:])

            # ---- output projection ----
            # ypart[ic] = fc 0..3 partial in f32 SBUF (fillers during heads
            # 8..11); tail adds fc 4..5 from PSUM.
            ypart = [apool.tile([128, D], f32, tag=f"yp{ic}", name=f"yp{ic}") for ic in range(IC)]

            def gen_outproj_partial(ic):
                for s, e in vsplits:
                    ps = half_ps(f"yh{ic}_{s}")
                    if has_bout:
                        yield nc.tensor.matmul(ps[:, 0:e - s], ones_t[:],
                                               bo_t[:, s:e], start=True, stop=False)
                    for fc in range(4):
                        yield nc.tensor.matmul(
                            ps[:, 0:e - s],
                            otn[fc][:, ic * 128:(ic + 1) * 128],
                            wo[fc][:, s:e],
                            start=(fc == 0 and not has_bout),
                            stop=(fc == 3))
                    nc.vector.tensor_copy(ypart[ic][:, s:e], ps[:, 0:e - s])

            def outproj_tail_mm(ic, ps, fc):
                for s, e in vsplits:
                    nc.tensor.matmul(
                        ps[:, s:e],
                        otn[fc][:, ic * 128:(ic + 1) * 128],
                        wo[fc][:, s:e],
                        start=(fc == 4), stop=(fc == 5))

            def outproj_finish(ic, ps):
                ysb = ypool.tile([128, D], f32, tag="y", name=f"y{ic}")
                nc.vector.tensor_tensor(
                    out=ysb[:], in0=ps[:, 0:D], in1=ypart[ic][:],
                    op=mybir.AluOpType.add)
                eng = nc.sync if ic % 2 == 0 else nc.scalar
                eng.dma_start(out=out_ext[ic * 128:(ic + 1) * 128, :], in_=ysb[:])

            # ---- phase A: q/k chunks for heads 0,1, then v chunks with
            # head-0 S/exp interleaved so ACT starts ~8us earlier ----
            tags = ["A", "B", "C"]
            ets = {}
            ots = {}

            def emit_s(step):
                h, kc = divmod(step, KC)
                sps = big_ps("A" if step % 2 == 0 else "B", f"sps{h}_{kc}")
                smm(h, kc, sps)
                et = espool.tile([128, N], bf16, tag="es", name=f"es{h}_{kc}")
                nc.scalar.activation(et[:], sps[:], Exp, scale=SCALE)
                ets[step] = et

            qk_chunk_full(0, "A")
            qk_chunk_full(DC, "B")
            for ic in range(6):
                v_chunk(ic, "C")
                if ic >= 1:
                    emit_s(ic - 1)    # S(0, 0..4) between v chunks

            # filler queue in deadline order: v[6] (step 7), v[7] (step 8),
            # qk chunks for heads 2,3 (step 16), heads 4..7 (step 32),
            # heads 8..11 (step 64); out-proj partials appended at h==7;
            # idempotent junk (qk recompute) only as clock-keeping backstop.
            fillers.append(gen_v_chunk(6))
            fillers.append(gen_v_chunk(7))
            for jc in [1, DC + 1, 2, DC + 2, 3, DC + 3, 4, DC + 4, 5, DC + 5]:
                fillers.append(gen_qk_chunk(jc))

            emitted = 5   # S(0,0..4) already issued during phase A
            for step in range(H * KC + 1):
                if step < H * KC and step >= emitted:
                    emit_s(step)
                    emitted = step + 1
                boundary = False
                if step > 0:
                    hp, kcp = divmod(step - 1, KC)
                    if kcp == 0:
                        ots[hp] = big_ps("C", f"ot{hp}")
                    avmm(hp, kcp, ets.pop(step - 1), ots[hp])
                    if kcp == KC - 1:
                        boundary = True
                        head_epilogue(hp, ots.pop(hp))
                        if hp % 2 == 1:
                            normalize_pair(hp // 2)
                        if hp == 7:
                            for ic in range(IC):
                                fillers.append(gen_outproj_partial(ic))
                        # pre-emit the next step's S so ACT keeps cadence
                        # while boundary fillers cover the ot-drain latency
                        if step + 1 < H * KC and step + 1 >= emitted:
                            emit_s(step + 1)
                            emitted = step + 2
                fill(10 if boundary else
                     (4 if step < 12 else (2 if step < 32 else 1)))

            # tail: fc4 matmuls (gated only on otn[4]) start during the
            # last pair's normalize chain; fc5 + finish follow per ic.
            tps = {}
            for ic in range(3):
                tps[ic] = big_ps(tags[ic % 3], f"yt{ic}")
                outproj_tail_mm(ic, tps[ic], 4)
            fill(10 ** 9)  # flush any remaining fillers
            for ic in range(3, IC):
                tps[ic] = big_ps(tags[ic % 3], f"yt{ic}")
                outproj_tail_mm(ic, tps[ic], 4)
                outproj_tail_mm(ic - 3, tps[ic - 3], 5)
                outproj_finish(ic - 3, tps.pop(ic - 3))
            for ic in (IC - 3, IC - 2, IC - 1):
                outproj_tail_mm(ic, tps[ic], 5)
                outproj_finish(ic, tps.pop(ic))

    nc.compile()
    return nc


# host-side W-column permutation within each 128-feature chunk:
# [A feats 0-31 | B feats 0-31 | A feats 32-63 | B feats 32-63]
_PERM128 = np.concatenate([np.arange(0, 32), np.arange(64, 96),
                           np.arange(32, 64), np.arange(96, 128)])
_QK_PERM = np.concatenate([jc * 128 + _PERM128 for jc in range(2 * DC)])


def _prepare(x, W_qkv, b_qkv, W_out, b_out):
    """Build the compiled graph and per-core input maps."""
    bf = ml_dtypes.bfloat16
    f8 = ml_dtypes.float8_e4m3
    xTf32 = np.ascontiguousarray(np.transpose(x, (0, 2, 1)))             # [B, D, N] f32
    xT = xTf32.astype(bf)
    wqkvT = np.ascontiguousarray(W_qkv.T)                                # [D, 3D]
    wvT = np.ascontiguousarray(wqkvT[:, 2 * D:]).astype(bf)              # [D, D]
    woutT = np.ascontiguousarray(W_out.T).astype(bf)                     # [D, D]
    # fp8 double-row packs [128, 2, *]: pair rows 256i+128j+r; the x side
    # ships main + residual so its quantization noise cancels.
    x8 = xTf32.astype(f8)                                                # [B, D, N]
    x8r = (xTf32 - x8.astype(np.float32)).astype(f8)
    w8 = wqkvT[:, :2 * D][:, _QK_PERM].astype(f8)                        # [D, 2D] permuted

    def packx(xq):
        return [np.concatenate([xq[:, 256 * i:256 * i + 128, :],
                                xq[:, 256 * i + 128:256 * i + 256, :]], axis=2)
                for i in range(NI)]                                      # [B, 128, 2N]

    xTf = packx(x8)
    xTr = packx(x8r)
    wqf = [np.ascontiguousarray(np.concatenate(
              [w8[256 * i:256 * i + 128, :], w8[256 * i + 128:256 * i + 256, :]],
              axis=1)) for i in range(NI)]                               # [128, 4D]
    has_bqkv = bool(np.any(b_qkv != 0))
    has_bout = bool(np.any(b_out != 0))

    nc = _build(has_bqkv, has_bout)

    in_maps = []
    for c in range(NCORES):
        m = {"xT": xT[c], "wvT": wvT, "woutT": woutT}
        for i in range(NI):
            m[f"xTf{i}"] = np.ascontiguousarray(xTf[i][c])
            m[f"xTr{i}"] = np.ascontiguousarray(xTr[i][c])
            m[f"wqf{i}"] = wqf[i]
        if has_bqkv:
            m["bqkv"] = np.ascontiguousarray(
                b_qkv[:2 * D][_QK_PERM]).astype(np.float32)
            m["bv16"] = np.ascontiguousarray(b_qkv[2 * D:]).astype(bf)
        if has_bout:
            m["bout16"] = np.ascontiguousarray(b_out).astype(bf)
        in_maps.append(m)
    return nc, in_maps


def kernel(x, W_qkv, b_qkv, W_out, b_out):
    from concourse.bass_utils import run_bass_kernel_spmd

    nc, in_maps = _prepare(x, W_qkv, b_qkv, W_out, b_out)

    res = None
    for attempt in range(3):
        try:
            res = run_bass_kernel_spmd(nc, in_maps, core_ids=list(range(NCORES)))
            break
        except Exception:
            if attempt == 2:
                raise
    return np.stack([res.results[c]["out"] for c in range(NCORES)], axis=0)



# revision 21
# speedup vs baseline: 1.0614x; 1.0614x over previous
"""Multi-head attention kernel for Trainium2, data-parallel over batch on 8 NeuronCores.

Reference computation (per batch element b of 8):
    qkv = x @ W_qkv.T + b_qkv            [1024, 2304]
    q, k, v = split(qkv)                 each [1024, 768], 12 heads x 64
    S_h = q_h @ k_h.T * d**-0.5          [1024, 1024] per head
    A_h = softmax(S_h, axis=-1)
    o_h = A_h @ v_h                      [1024, 64]
    y = concat(o) @ W_out.T + b_out      [1024, 768]

Strategy: one batch element per core (zero communication). All attention
matmuls (S, A@V) and the v/out projections run in bf16 with f32 PSUM
accumulation. The q/k projection runs in fp8e4m3 DoubleRow: two 128-row
k-subtiles contract per instruction, halving the instruction count for its
K=768 contraction (measured: a DoubleRow matmul costs the same per output
column as bf16, so fp8 only pays where it deepens contraction — it would NOT
speed up the K=64 scores matmul). fp8 on the q/k path is safe: quantization
noise reaches the output only through softmax weight perturbations (~1%),
while v/A/out-proj quantization would hit the output linearly, so those stay
bf16.

Scores are computed transposed (S^T[j,i] keys-on-partition) so exp(S^T) feeds
A@V as the moving operand with V stationary. A ones-column appended to V
yields softmax denominators free. Softmax max-subtraction is skipped:
scores*scale are O(1) (std ~0.14); f32 exp cannot overflow below inputs of 88.

Scheduling: ACT exp takes ~1.1us per [128,1024] tile (96 tiles); the PE has
slightly more total work, so both engines must stay saturated. Attention runs
as one flat software pipeline over (head, kc): S(step+1) issues before
A@V(step) so the PE never waits on exp. PE bubbles take real work from a
filler queue (remaining v chunks, later q/k chunks, partial output projection
fc 0..3 flushed to SBUF f32). Extra fillers are pumped at head boundaries to
cover the head-drain DVE latency — a ~1us PE stall measurably drops the PE
DVFS clock from 2.4 GHz to ~1.2 GHz for tens of microseconds, which is the
dominant failure mode. An idempotent junk-work generator backstops the queue.
The denominator row is staged PSUM->SBUF before reciprocal_approx_fast (the
custom DVE op misreads PSUM rows at partition base 64); reciprocals stay f32
end-to-end. Normalize multiplies run on DVE (GpSimd TensorTensor measured
2.1us vs DVE 0.6us for [64,1024]).
"""

import numpy as np
import ml_dtypes

B, N, D, H, HD = 8, 1024, 768, 12, 64
NCORES = 8
SCALE = float(D) ** -0.5
DC = D // 128            # 6 chunks of 128 for d=768
IC = N // 128            # 8 token chunks
KC = N // 128            # 8 key chunks
NI = 3                   # 256-wide contraction double-chunks for fp8 qk-proj


def _build(has_bqkv: bool, has_bout: bool):
    import concourse.bass as bass
    import concourse.mybir as mybir
    import concourse.tile as tile
    from concourse import bacc

    f32 = mybir.dt.float32
    bf16 = mybir.dt.bfloat16
    fp8 = mybir.dt.float8e4
    Exp = mybir.ActivationFunctionType.Exp
    DoubleRow = mybir.MatmulPerfMode.DoubleRow

    nc = bacc.Bacc("TRN2", target_bir_lowering=False, debug=False,
                   num_devices=NCORES)

    xT_ext = nc.dram_tensor("xT", [D, N], bf16, kind="ExternalInput")
    xTf_ext = [nc.dram_tensor(f"xTf{i}", [128, 2 * N], fp8, kind="ExternalInput")
               for i in range(NI)]
    xTr_ext = [nc.dram_tensor(f"xTr{i}", [128, 2 * N], fp8, kind="ExternalInput")
               for i in range(NI)]
    wqf_ext = [nc.dram_tensor(f"wqf{i}", [128, 4 * D], fp8, kind="ExternalInput")
               for i in range(NI)]
    wvT_ext = nc.dram_tensor("wvT", [D, D], bf16, kind="ExternalInput")
    woutT_ext = nc.dram_tensor("woutT", [D, D], bf16, kind="ExternalInput")
    if has_bqkv:
        bqkv_ext = nc.dram_tensor("bqkv", [2 * D], f32, kind="ExternalInput")
        bv16_ext = nc.dram_tensor("bv16", [D], bf16, kind="ExternalInput")
    if has_bout:
        bout16_ext = nc.dram_tensor("bout16", [D], bf16, kind="ExternalInput")
    out_ext = nc.dram_tensor("out", [N, D], f32, kind="ExternalOutput")
    recip_dram = nc.dram_tensor("recip_scratch", [H, N], f32)

    with tile.TileContext(nc) as tc:
        with (
            tc.tile_pool(name="w", bufs=1) as wpool,
            tc.tile_pool(name="act", bufs=1) as apool,
            tc.tile_pool(name="es", bufs=7) as espool,
            tc.tile_pool(name="rows", bufs=2) as rowpool,
            tc.tile_pool(name="bc", bufs=3) as bcpool,
            tc.tile_pool(name="y", bufs=3) as ypool,
            tc.tile_pool(name="ps", bufs=1, space="PSUM") as pspool,
        ):
            # ---- input DMAs: qk-proj operands first for earliest PE start ----
            xTf = [wpool.tile([128, 2 * N], fp8, tag=f"xTf{i}", name=f"xTf{i}") for i in range(NI)]
            xTr = [wpool.tile([128, 2 * N], fp8, tag=f"xTr{i}", name=f"xTr{i}") for i in range(NI)]
            wqf = [wpool.tile([128, 4 * D], fp8, tag=f"wqf{i}", name=f"wqf{i}") for i in range(NI)]
            xT = [wpool.tile([128, N], bf16, tag=f"xT{i}", name=f"xT{i}") for i in range(DC)]
            wv = [wpool.tile([128, D], bf16, tag=f"wv{i}", name=f"wv{i}") for i in range(DC)]
            wo = [wpool.tile([128, D], bf16, tag=f"wo{i}", name=f"wo{i}") for i in range(DC)]
            for i in range(NI):
                nc.scalar.dma_start(out=xTf[i][:], in_=xTf_ext[i][:, :])
                nc.sync.dma_start(out=wqf[i][:], in_=wqf_ext[i][:, :])
            for i in range(NI):
                nc.gpsimd.dma_start(out=xTr[i][:], in_=xTr_ext[i][:, :])
            for dc in range(DC):
                nc.scalar.dma_start(out=xT[dc][:], in_=xT_ext[dc * 128:(dc + 1) * 128, :])
                nc.sync.dma_start(out=wv[dc][:], in_=wvT_ext[dc * 128:(dc + 1) * 128, :])
            for dc in range(DC):
                nc.scalar.dma_start(out=wo[dc][:], in_=woutT_ext[dc * 128:(dc + 1) * 128, :])

            # PSUM budget (8 banks): A,B = sps double-buffer (2+2), C = ot (2),
            # D = two [128,512] one-bank tiles for warmup/filler projections.
            def big_ps(tag, name):
                return pspool.tile([128, N], f32, tag=tag, name=name)

            def half_ps(name):
                return pspool.tile([128, 512], f32, tag="D", bufs=2, name=name)

            if has_bqkv:
                bqk_t = wpool.tile([128, 2 * DC], f32, tag="bqk")
                for jc in range(2 * DC):
                    nc.sync.dma_start(
                        out=bqk_t[:, jc:jc + 1],
                        in_=bqkv_ext[jc * 128:(jc + 1) * 128][:, None])
                bv_t = wpool.tile([1, D], bf16, tag="bv")
                nc.sync.dma_start(out=bv_t[:], in_=bv16_ext[:][None, :])
            if has_bout:
                bo_t = wpool.tile([1, D], bf16, tag="bo")
                nc.sync.dma_start(out=bo_t[:], in_=bout16_ext[:][None, :])
            if has_bqkv or has_bout:
                ones_t = wpool.tile([1, 128], bf16, tag="ones")
                nc.vector.memset(ones_t[:], 1.0)

            xTf3 = [t.rearrange("p (two n) -> p two n", two=2) for t in xTf]
            xTr3 = [t.rearrange("p (two n) -> p two n", two=2) for t in xTr]
            wqf3 = [t.rearrange("p (two n) -> p two n", two=2) for t in wqf]

            # ---- q^T/k^T fp8 DoubleRow tiles. Host W-permutation puts
            # [A0-31|B0-31|A32-63|B32-63] on each chunk's partitions, so the
            # DR [32p x 2slot] pack is one cast + one partition-shift DMA.
            qk8 = [apool.tile([128, 2, N], fp8, tag=f"qk8_{j}", name=f"qk8_{j}")
                   for j in range(2 * DC)]

            def qk_store(jc, src, s=0, e=N):
                t = qk8[jc]
                if has_bqkv:
                    nc.vector.tensor_scalar_add(t[:, 0, s:e], src,
                                                bqk_t[:, jc:jc + 1])
                else:
                    nc.vector.tensor_copy(t[:, 0, s:e], src)
                # slot-1 shift stays on DVE: a DMA here would put a slow
                # DMA-queue semaphore on every S matmul's (coarse) wait set.
                nc.vector.tensor_copy(t[0:64, 1, s:e], t[64:128, 0, s:e])

            def qk_chunk_full(jc, tag):
                """q/k projection chunk jc, fp8 DoubleRow + x-residual."""
                ps = big_ps(tag, f"qkps{jc}")
                for ih in range(2):
                    for i in range(NI):
                        for x3 in (xTf3, xTr3):
                            nc.tensor.matmul(
                                ps[:, ih * 512:(ih + 1) * 512],
                                wqf3[i][:, :, jc * 128:(jc + 1) * 128],
                                x3[i][:, :, ih * 512:(ih + 1) * 512],
                                start=(i == 0 and x3 is xTf3),
                                stop=(i == NI - 1 and x3 is xTr3),
                                perf_mode=DoubleRow)
                qk_store(jc, ps[:, 0:N])

            def gen_qk_chunk(jc):
                """Filler generator: one chunk as 2 half-bank units."""
                for ih in range(2):
                    ps = half_ps(f"qkh{jc}_{ih}")
                    for i in range(NI):
                        for x3 in (xTf3, xTr3):
                            yield nc.tensor.matmul(
                                ps[:], wqf3[i][:, :, jc * 128:(jc + 1) * 128],
                                x3[i][:, :, ih * 512:(ih + 1) * 512],
                                start=(i == 0 and x3 is xTf3),
                                stop=(i == NI - 1 and x3 is xTr3),
                                perf_mode=DoubleRow)
                    qk_store(jc, ps[:], s=ih * 512, e=(ih + 1) * 512)

            # ---- v : [1024 tokens, 12 heads x (64+1)] with ones column ----
            v = [apool.tile([128, H, HD + 1], bf16, tag=f"v{i}", name=f"v{i}") for i in range(IC)]
            for ic in range(IC):
                nc.vector.memset(v[ic][:, :, HD:HD + 1], 1.0)
            vsplits = [(0, 512), (512, 768)]

            def v_chunk(ic, tag):
                ps = big_ps(tag, f"vps{ic}")
                if has_bqkv:
                    for s, e in vsplits:
                        nc.tensor.matmul(ps[:, s:e], ones_t[:],
                                         bv_t[:, s:e], start=True, stop=False)
                for s, e in vsplits:
                    for dc in range(DC):
                        nc.tensor.matmul(
                            ps[:, s:e],
                            xT[dc][:, ic * 128:(ic + 1) * 128],
                            wv[dc][:, s:e],
                            start=(dc == 0 and not has_bqkv), stop=(dc == DC - 1))
                nc.vector.tensor_copy(
                    v[ic][:, :, 0:HD],
                    ps[:, 0:D].rearrange("p (h e) -> p h e", h=H))

            def gen_v_chunk(ic):
                """half-bank filler version: heads 0..7 then 8..11."""
                for s, e in vsplits:
                    ps = half_ps(f"vh{ic}_{s}")
                    if has_bqkv:
                        yield nc.tensor.matmul(ps[:, 0:e - s], ones_t[:],
                                               bv_t[:, s:e], start=True, stop=False)
                    for dc in range(DC):
                        yield nc.tensor.matmul(
                            ps[:, 0:e - s],
                            xT[dc][:, ic * 128:(ic + 1) * 128],
                            wv[dc][:, s:e],
                            start=(dc == 0 and not has_bqkv), stop=(dc == DC - 1))
                    h0, h1 = s // HD, e // HD
                    nc.vector.tensor_copy(
                        v[ic][:, h0:h1, 0:HD],
                        ps[:, 0:e - s].rearrange("p (h e) -> p h e", h=h1 - h0))


            # ---- attention: flat software pipeline over (head, kc) ----
            # otu: unnormalized o^T + denominator row per head, f32, short-lived
            otn = [apool.tile([128, N], bf16, tag=f"otn{t}", name=f"otn{t}") for t in range(DC)]
            otu = {}

            fillers = []

            def fill(n):
                while n > 0 and fillers:
                    try:
                        next(fillers[0])
                        n -= 1
                    except StopIteration:
                        fillers.pop(0)

            def smm(h, kc, sps):
                qt, kt = qk8[h // 2], qk8[DC + h // 2]
                p0 = 32 * (h % 2)
                for ih in range(2):
                    nc.tensor.matmul(
                        sps[:, ih * 512:(ih + 1) * 512],
                        kt[p0:p0 + 32, :, kc * 128:(kc + 1) * 128],
                        qt[p0:p0 + 32, :, ih * 512:(ih + 1) * 512],
                        start=True, stop=True, perf_mode=DoubleRow)

            def avmm(h, kc, et, ot):
                for ih in range(2):
                    nc.tensor.matmul(
                        ot[0:HD + 1, ih * 512:(ih + 1) * 512],
                        v[kc][:, h, :],
                        et[:, ih * 512:(ih + 1) * 512],
                        start=(kc == 0), stop=(kc == KC - 1))

            def head_epilogue(h, ot):
                # recip path first (it gates normalize/out-proj); the custom
                # DVE op misreads rows at partition base 64, so stage to a
                # base-0 SBUF row before reciprocal.
                drow = rowpool.tile([1, N], f32, tag="drow", bufs=2, name=f"drow{h}")
                nc.vector.tensor_copy(drow[:], ot[HD:HD + 1, :])
                rc32 = rowpool.tile([1, N], f32, tag="rc32", bufs=2, name=f"rc32_{h}")
                nc.vector.reciprocal_approx_fast(rc32[:], drow[:])
                eng = nc.sync if h % 2 == 0 else nc.scalar
                eng.dma_start(out=recip_dram[h:h + 1, :], in_=rc32[:])
                otu[h] = rowpool.tile([HD + 1, N], f32, tag="otu", bufs=3,
                                      name=f"otu{h}")
                nc.vector.tensor_copy(otu[h][:], ot[:HD + 1, :])

            def normalize_pair(t):
                for i in range(2):
                    h = 2 * t + i
                    bc = bcpool.tile([64, N], f32, tag="bc", bufs=3, name=f"bc{h}")
                    eng = nc.sync if h % 2 == 0 else nc.scalar
                    eng.dma_start(
                        out=bc[:],
                        in_=recip_dram[h:h + 1, :].to_broadcast((64, N)))
                    with nc.allow_low_precision(reason="bf16 normalized o; 2e-2 gate"):
                        nc.vector.tensor_mul(otn[t][64 * i:64 * i + 64, :],
                                             otu.pop(h)[0:HD, :], bc[:])

            # ---- output projection ----
            # ypart[ic] = fc 0..3 partial in f32 SBUF (fillers during heads
            # 8..11); tail adds fc 4..5 from PSUM.
            ypart = [apool.tile([128, D], f32, tag=f"yp{ic}", name=f"yp{ic}") for ic in range(IC)]

            def gen_outproj_partial(ic):
                for s, e in vsplits:
                    ps = half_ps(f"yh{ic}_{s}")
                    if has_bout:
                        yield nc.tensor.matmul(ps[:, 0:e - s], ones_t[:],
                                               bo_t[:, s:e], start=True, stop=False)
                    for fc in range(4):
                        yield nc.tensor.matmul(
                            ps[:, 0:e - s],
                            otn[fc][:, ic * 128:(ic + 1) * 128],
                            wo[fc][:, s:e],
                            start=(fc == 0 and not has_bout),
                            stop=(fc == 3))
                    nc.vector.tensor_copy(ypart[ic][:, s:e], ps[:, 0:e - s])

            def outproj_tail_mm(ic, ps, fc):
                for s, e in vsplits:
                    nc.tensor.matmul(
                        ps[:, s:e],
                        otn[fc][:, ic * 128:(ic + 1) * 128],
                        wo[fc][:, s:e],
                        start=(fc == 4), stop=(fc == 5))

            def outproj_finish(ic, ps):
                ysb = ypool.tile([128, D], f32, tag="y", name=f"y{ic}")
                nc.vector.tensor_tensor(
                    out=ysb[:], in0=ps[:, 0:D], in1=ypart[ic][:],
                    op=mybir.AluOpType.add)
                eng = nc.sync if ic % 2 == 0 else nc.scalar
                eng.dma_start(out=out_ext[ic * 128:(ic + 1) * 128, :], in_=ysb[:])

            # ---- phase A: q/k chunks for heads 0,1, then v chunks with
            # head-0 S/exp interleaved so ACT starts ~8us earlier ----
            tags = ["A", "B", "C"]
            ets = {}
            ots = {}

            def emit_s(step):
                h, kc = divmod(step, KC)
                sps = big_ps("A" if step % 2 == 0 else "B", f"sps{h}_{kc}")
                smm(h, kc, sps)
                et = espool.tile([128, N], bf16, tag="es", name=f"es{h}_{kc}")
                nc.scalar.activation(et[:], sps[:], Exp, scale=SCALE)
                ets[step] = et

            qk_chunk_full(0, "A")
            qk_chunk_full(DC, "B")
            for ic in range(6):
                v_chunk(ic, "C")
                if ic >= 1:
                    emit_s(ic - 1)    # S(0, 0..4) between v chunks

            # filler queue in deadline order: v[6] (step 7), v[7] (step 8),
            # qk chunks for heads 2,3 (step 16), heads 4..7 (step 32),
            # heads 8..11 (step 64); out-proj partials appended at h==7;
            # idempotent junk (qk recompute) only as clock-keeping backstop.
            fillers.append(gen_v_chunk(6))
            fillers.append(gen_v_chunk(7))
            for jc in [1, DC + 1, 2, DC + 2, 3, DC + 3, 4, DC + 4, 5, DC + 5]:
                fillers.append(gen_qk_chunk(jc))

            emitted = 5   # S(0,0..4) already issued during phase A
            for step in range(H * KC + 1):
                if step < H * KC and step >= emitted:
                    emit_s(step)
                    emitted = step + 1
                boundary = False
                if step > 0:
                    hp, kcp = divmod(step - 1, KC)
                    if kcp == 0:
                        ots[hp] = big_ps("C", f"ot{hp}")
                    avmm(hp, kcp, ets.pop(step - 1), ots[hp])
                    if kcp == KC - 1:
                        boundary = True
                        head_epilogue(hp, ots.pop(hp))
                        if hp % 2 == 1:
                            normalize_pair(hp // 2)
                        if hp == 7:
                            for ic in range(IC):
                                fillers.append(gen_outproj_partial(ic))
                        # pre-emit the next step's S so ACT keeps cadence
                        # while boundary fillers cover the ot-drain latency
                        if step + 1 < H * KC and step + 1 >= emitted:
                            emit_s(step + 1)
                            emitted = step + 2
                fill(10 if boundary else
                     (4 if step < 12 else (2 if step < 32 else 1)))

            # tail: fc4 matmuls (gated only on otn[4]) start during the
            # last pair's normalize chain; fc5 + finish follow per ic.
            tps = {}
            for ic in range(3):
                tps[ic] = big_ps(tags[ic % 3], f"yt{ic}")
                outproj_tail_mm(ic, tps[ic], 4)
            fill(10 ** 9)  # flush any remaining fillers
            for ic in range(3, IC):
                tps[ic] = big_ps(tags[ic % 3], f"yt{ic}")
                outproj_tail_mm(ic, tps[ic], 4)
                outproj_tail_mm(ic - 3, tps[ic - 3], 5)
                outproj_finish(ic - 3, tps.pop(ic - 3))
            for ic in (IC - 3, IC - 2, IC - 1):
                outproj_tail_mm(ic, tps[ic], 5)
                outproj_finish(ic, tps.pop(ic))

    nc.compile()
    return nc


# host-side W-column permutation within each 128-feature chunk:
# [A feats 0-31 | B feats 0-31 | A feats 32-63 | B feats 32-63]
_PERM128 = np.concatenate([np.arange(0, 32), np.arange(64, 96),
                           np.arange(32, 64), np.arange(96, 128)])
_QK_PERM = np.concatenate([jc * 128 + _PERM128 for jc in range(2 * DC)])


def _prepare(x, W_qkv, b_qkv, W_out, b_out):
    """Build the compiled graph and per-core input maps."""
    bf = ml_dtypes.bfloat16
    f8 = ml_dtypes.float8_e4m3
    xTf32 = np.ascontiguousarray(np.transpose(x, (0, 2, 1)))             # [B, D, N] f32
    xT = xTf32.astype(bf)
    wqkvT = np.ascontiguousarray(W_qkv.T)                                # [D, 3D]
    wvT = np.ascontiguousarray(wqkvT[:, 2 * D:]).astype(bf)              # [D, D]
    woutT = np.ascontiguousarray(W_out.T).astype(bf)                     # [D, D]
    # fp8 double-row packs [128, 2, *]: pair rows 256i+128j+r; the x side
    # ships main + residual so its quantization noise cancels.
    x8 = xTf32.astype(f8)                                                # [B, D, N]
    x8r = (xTf32 - x8.astype(np.float32)).astype(f8)
    w8 = wqkvT[:, :2 * D][:, _QK_PERM].astype(f8)                        # [D, 2D] permuted

    def packx(xq):
        return [np.concatenate([xq[:, 256 * i:256 * i + 128, :],
                                xq[:, 256 * i + 128:256 * i + 256, :]], axis=2)
                for i in range(NI)]                                      # [B, 128, 2N]

    xTf = packx(x8)
    xTr = packx(x8r)
    wqf = [np.ascontiguousarray(np.concatenate(
              [w8[256 * i:256 * i + 128, :], w8[256 * i + 128:256 * i + 256, :]],
              axis=1)) for i in range(NI)]                               # [128, 4D]
    has_bqkv = bool(np.any(b_qkv != 0))
    has_bout = bool(np.any(b_out != 0))

    nc = _build(has_bqkv, has_bout)

    in_maps = []
    for c in range(NCORES):
        m = {"xT": xT[c], "wvT": wvT, "woutT": woutT}
        for i in range(NI):
            m[f"xTf{i}"] = np.ascontiguousarray(xTf[i][c])
            m[f"xTr{i}"] = np.ascontiguousarray(xTr[i][c])
            m[f"wqf{i}"] = wqf[i]
        if has_bqkv:
            m["bqkv"] = np.ascontiguousarray(
                b_qkv[:2 * D][_QK_PERM]).astype(np.float32)
            m["bv16"] = np.ascontiguousarray(b_qkv[2 * D:]).astype(bf)
        if has_bout:
            m["bout16"] = np.ascontiguousarray(b_out).astype(bf)
        in_maps.append(m)
    return nc, in_maps


def kernel(x, W_qkv, b_qkv, W_out, b_out):
    from concourse.bass_utils import run_bass_kernel_spmd

    nc, in_maps = _prepare(x, W_qkv, b_qkv, W_out, b_out)

    res = None
    for attempt in range(3):
        try:
            res = run_bass_kernel_spmd(nc, in_maps, core_ids=list(range(NCORES)))
            break
        except Exception:
            if attempt == 2:
                raise
    return np.stack([res.results[c]["out"] for c in range(NCORES)], axis=0)

